# revision 7
# baseline (speedup 1.0000x reference)
"""Trainium2 Bass kernel: 2-layer KNN message passing (AccumulateKnn MLP).

out = concat([L0, L1, x]); per layer: f = relu(feats@W+b) (b==0 required),
L = concat(w*mean_k f[idx], w*max_k f[idx]) - concat(f, f), w = exp(-1).

Design (8 cores, SPMD, full inputs fed per core):
- Vertices sharded (25000/core) and host-sorted per shard by window profile
  to reduce gather padding; outputs un-permuted on host.
- Feature tables laid out as 8 rank blocks ("windows") of SHARD_PAD rows.
  Layer-1 tables f0w[w] are computed redundantly on every core from a
  host-transposed, host-permuted x_T (per-window DRAM tiles so gathers can
  begin before the full dense phase finishes). Layer-2 table f1full is
  AllGathered from per-core shards.
- Gather calls are merged across GT=4 vertex tiles: one gpsimd.dma_gather
  per (group, window) rotating 4 SWDGE queues. This amortizes the ~1-2.4us
  fixed SWDGE descriptor-generation cost per call that dominated the
  per-(tile, window) version (GpSimd engine was 84% busy).
- Reduction per (tile, window): one strided DVE tensor_reduce (axis X over
  the J gathered columns) into dense per-tile slots, then one final reduce
  over slots, then scalar_tensor_tensor fuses the 1/K mean scale with the
  self-feature subtraction. PE does the layer-2 dense via transpose+matmul.
"""

import numpy as np

import concourse.bacc as bacc
import concourse.tile as tile
from concourse import mybir
from concourse.bass_utils import run_bass_kernel_spmd
from concourse.masks import make_identity

W_EXP = float(np.exp(-1.0))
NCORES = 8
P = 128
GA = 512   # vertices per dense-phase group
GT = 4     # vertex tiles (of 128) per merged gather call group
dt = mybir.dt


def _ceil_to(x, m):
    return (x + m - 1) // m * m


class Cfg:
    def __init__(self, n=200000, k=32, f=64):
        assert f == 64 and k == 32
        self.n, self.k, self.f = n, k, f
        assert n % NCORES == 0
        self.shard = n // NCORES
        # rank block / window size; +1 guarantees zero pad rows exist
        self.spad = _ceil_to(self.shard + 1, GA)
        assert self.spad < 32768
        assert self.spad % (P * GT) == 0
        self.tiles = self.spad // P
        self.groups = self.tiles // GT
        self.npad = self.spad * NCORES


def prep(cfg: Cfg, x, neighbour_indices, W0, b0, W1, b1):
    """Host prep: per-core sort, merged per-(group, window) index streams,
    inputs. Returns (in_maps, js, goffs, perms)."""
    assert np.allclose(b0, 0.0) and np.allclose(b1, 0.0), "nonzero bias unsupported"
    n, k, f, S, SP = cfg.n, cfg.k, cfg.f, cfg.shard, cfg.spad
    x = np.asarray(x, np.float32)
    idx = np.asarray(neighbour_indices, np.int64)
    w = np.float32(W_EXP)
    w0w = np.ascontiguousarray(np.asarray(W0, np.float32) * w)
    w1w = np.ascontiguousarray(np.asarray(W1, np.float32) * w)

    owner = (idx // S).astype(np.int32)  # window of each neighbour [n, k]
    local = (idx % S).astype(np.int32)

    # per-core sort of shard vertices by window profile (reduces padding):
    # (argmax window, its count, 2nd argmax window, its count)
    perms, invs = [], []
    for c in range(NCORES):
        m = np.zeros((S, NCORES), np.int32)
        ow = owner[c * S:(c + 1) * S]
        for wdw in range(NCORES):
            m[:, wdw] = (ow == wdw).sum(axis=1)
        a1 = m.argmax(1)
        mx1 = m.max(1)
        m2 = m.copy()
        m2[np.arange(S), a1] = -1
        a2 = m2.argmax(1)
        mx2 = m2.max(1)
        key = ((a1.astype(np.int64) * 64 + mx1) * 8 + a2) * 64 + mx2
        perm = np.argsort(key, kind="stable")  # sortedpos -> orig local
        inv = np.empty(S, np.int32)
        inv[perm] = np.arange(S, dtype=np.int32)
        perms.append(perm)
        invs.append(inv)

    # table-local position of each neighbour = inv-perm of its local id
    tloc = np.empty_like(local)
    for c in range(NCORES):
        sel = owner == c
        tloc[sel] = invs[c][local[sel]]

    # per-core sorted neighbour lists grouped by window (vectorized)
    PADIDX = SP - 1  # zero row
    sorted_t = np.zeros((NCORES, SP, k), np.int32)
    m_all = np.zeros((NCORES, SP, NCORES), np.int32)
    start_all = np.zeros((NCORES, SP, NCORES), np.int32)
    for c in range(NCORES):
        sl = slice(c * S, (c + 1) * S)
        ow_s = owner[sl][perms[c]]  # [S, k] sorted vertex order
        tl_s = tloc[sl][perms[c]]
        ord2 = np.argsort(ow_s, axis=1, kind="stable")
        sorted_t[c, :S] = np.take_along_axis(tl_s, ord2, axis=1)
        for wdw in range(NCORES):
            m_all[c, :S, wdw] = (ow_s == wdw).sum(axis=1)
        start_all[c] = np.concatenate(
            [np.zeros((SP, 1), np.int32), np.cumsum(m_all[c], axis=1)[:, :-1]],
            axis=1)

    # shared per-(tile, window) rectangle heights
    mt = m_all.reshape(NCORES, cfg.tiles, P, NCORES)
    js = mt.max(axis=2).max(axis=0)  # [tiles, NCORES]
    # per-call (group, window) column counts; carveout limit check
    jg = js.reshape(cfg.groups, GT, NCORES).sum(axis=1)  # [groups, NCORES]
    assert jg.max() * P <= 16000, f"gather call too large: {jg.max()*P} idxs"
    # int16-stream column offsets per group (all windows of a group adjacent)
    goffs = np.concatenate([[0], np.cumsum(jg.sum(axis=1) * 8)]).astype(np.int64)

    # build wrapped int16 streams [128, total_cols] per core, ordered
    # (group, window, tile-within-group)
    streams = []
    jmax = int(js.max())
    jr = np.arange(max(jmax, 1))
    for c in range(NCORES):
        blocks = []
        for g in range(cfg.groups):
            for wdw in range(NCORES):
                for tt in range(GT):
                    t = g * GT + tt
                    J = int(js[t, wdw])
                    if J == 0:
                        continue
                    r = slice(t * P, (t + 1) * P)
                    starts = start_all[c, r, wdw][:, None]  # [P, 1]
                    mm = m_all[c, r, wdw][:, None]
                    src = np.clip(starts + jr[None, :J], 0, k - 1)
                    vals = np.take_along_axis(sorted_t[c, r], src, axis=1)
                    vals = np.where(jr[None, :J] < mm, vals, PADIDX)  # [P, J]
                    flat = vals.T.reshape(J * P).astype(np.int16)
                    wr = flat.reshape(J * P // 16, 16).T  # [16, J*8]
                    blocks.append(np.tile(wr, (8, 1)))
        streams.append(np.ascontiguousarray(np.concatenate(blocks, axis=1)))
    assert streams[0].shape[1] == goffs[-1]

    # x_T in table order (all cores' sorted shards, zero-padded blocks)
    xt = np.zeros((f, cfg.npad), np.float32)
    for c in range(NCORES):
        xt[:, c * SP: c * SP + S] = x[c * S:(c + 1) * S][perms[c]].T

    in_maps = []
    for c in range(NCORES):
        xts = np.zeros((f, SP), np.float32)
        xts[:, :S] = x[c * S:(c + 1) * S][perms[c]].T
        in_maps.append({
            "xt": xt,
            "xts": np.ascontiguousarray(xts),
            "idxw": streams[c],
            "w0": w0w,
            "w1": w1w,
        })
    return in_maps, js, goffs, perms


def build_nc(cfg: Cfg, js, goffs):
    f, K, SP = cfg.f, cfg.k, cfg.spad
    total_cols = int(goffs[-1])
    jg = js.reshape(cfg.groups, GT, NCORES).sum(axis=1)
    nc = bacc.Bacc("TRN2", target_bir_lowering=False, debug=False,
                   enable_asserts=False, num_devices=NCORES,
                   num_swdge_queues=4)

    xt = nc.dram_tensor("xt", [f, cfg.npad], dt.float32, kind="ExternalInput")
    xts = nc.dram_tensor("xts", [f, SP], dt.float32, kind="ExternalInput")
    idxw = nc.dram_tensor("idxw", [P, total_cols], dt.int16, kind="ExternalInput")
    w0 = nc.dram_tensor("w0", [f, 64], dt.float32, kind="ExternalInput")
    w1 = nc.dram_tensor("w1", [128, 64], dt.float32, kind="ExternalInput")
    out01 = nc.dram_tensor("out01", [SP, 256], dt.float32, kind="ExternalOutput")

    qcnt = [0]
    add, mx, mult, sub = (mybir.AluOpType.add, mybir.AluOpType.max,
                          mybir.AluOpType.mult, mybir.AluOpType.subtract)
    X = mybir.AxisListType.X
    Relu = mybir.ActivationFunctionType.Relu
    Copy = mybir.ActivationFunctionType.Copy

    with tile.TileContext(nc) as tc:
        with (
            tc.tile_pool(name="dram", bufs=1, space="DRAM") as dram,
            tc.tile_pool(name="const", bufs=1) as const,
            tc.tile_pool(name="densea", bufs=3) as densea,
            tc.tile_pool(name="psa", bufs=2, space="PSUM") as psa,
            tc.tile_pool(name="gat", bufs=3) as gat,
            tc.tile_pool(name="idxp", bufs=3) as idxp,
            tc.tile_pool(name="red", bufs=3) as red,
            tc.tile_pool(name="pst", bufs=2, space="PSUM") as pst,
            tc.tile_pool(name="ps1", bufs=2, space="PSUM") as ps1,
        ):
            f0w = [dram.tile([SP, f], dt.float32, name=f"f0w{w}")
                   for w in range(NCORES)]
            f0self = dram.tile([SP, f], dt.float32, name="f0self")
            f1self = dram.tile([SP, f], dt.float32, name="f1self")
            f1in = dram.tile([SP, f], dt.float32, name="f1in")
            f1full = dram.tile([cfg.npad, f], dt.float32, name="f1full",
                               addr_space="Shared")

            dma_sems = [nc.alloc_semaphore(f"swdge_dma{q}") for q in range(4)]

            w0s = const.tile([f, 64], dt.float32)
            nc.sync.dma_start(out=w0s[:], in_=w0[:, :])
            w1s = const.tile([128, 64], dt.float32)
            nc.sync.dma_start(out=w1s[:], in_=w1[:, :])
            ident = const.tile([P, P], dt.float32)
            make_identity(nc, ident[:])

            def dense_chunk(src, col0, n_cols, out_table, row0, scale):
                for g in range(n_cols // GA):
                    v0 = g * GA
                    xt_t = densea.tile([f, GA], dt.float32, tag="xa")
                    nc.sync.dma_start(out=xt_t[:],
                                      in_=src[:, col0 + v0: col0 + v0 + GA])
                    ps = psa.tile([P, 256], dt.float32, tag="psa")
                    for m in range(4):
                        nc.tensor.matmul(
                            out=ps[:, m * 64:(m + 1) * 64],
                            lhsT=xt_t[:, m * P:(m + 1) * P],
                            rhs=w0s[:],
                            start=True, stop=True)
                    fa = densea.tile([P, 4, 64], dt.float32, tag="fa")
                    nc.scalar.activation(
                        out=fa[:].rearrange("p a b -> p (a b)"), in_=ps[:],
                        func=Relu, scale=scale)
                    nc.sync.dma_start(
                        out=out_table[row0 + v0: row0 + v0 + GA, :]
                            .rearrange("(m p) f -> p m f", p=P),
                        in_=fa[:])

            # layer-1 dense: per-window tables so gathers can start early
            for wdw in range(NCORES):
                dense_chunk(xt, wdw * SP, SP, f0w[wdw], 0, 1.0)
            dense_chunk(xts, 0, SP, f0self, 0, 1.0 / W_EXP)

            def gather_group(g, tables, self_tab, out_lo, produce_f1):
                jsg = js[g * GT:(g + 1) * GT]  # [GT, NCORES]
                cg = int(jg[g].sum())          # total idx cols this group
                off0 = int(goffs[g])
                idx_t = idxp.tile([P, cg * 8], dt.int16, tag="idx")
                nc.sync.dma_start(out=idx_t[:], in_=idxw[:, off0:off0 + cg * 8])

                self_g = red.tile([P, GT, 64], dt.float32, tag="self")
                nc.sync.dma_start(
                    out=self_g[:],
                    in_=self_tab[g * GT * P:(g + 1) * GT * P, :]
                        .rearrange("(m p) f -> p m f", p=P))
                out_g = red.tile([P, GT, 128], dt.float32, tag="out")
                ssum = red.tile([P, GT, NCORES, 64], dt.float32, tag="ssum")
                smax = red.tile([P, GT, NCORES, 64], dt.float32, tag="smax")

                # one gather per window covering all GT tiles; its per-tile
                # reduces follow immediately so the G buffer recycles fast
                sis = [0] * GT
                ioff = 0
                for wdw in range(NCORES):
                    C = int(jg[g, wdw])
                    if C == 0:
                        continue
                    Gt = gat.tile([P, C, f], dt.float32, tag="G")
                    nidx = C * P
                    q = qcnt[0] % 4
                    # prepare_only decouples Q7 descriptor generation from
                    # the DMA drain: the Pool engine is released after
                    # desc-gen; the transfer fires on trigger_dma and
                    # consumers wait on the prep's DMA-completion sem.
                    nc.gpsimd.dma_gather(
                        out_ap=Gt[:],
                        in_ap=tables[wdw],
                        idxs_ap=idx_t[:, ioff:ioff + C * 8],
                        num_idxs=nidx, num_idxs_reg=nidx, elem_size=f,
                        single_packet=False, queue_num=q,
                        prepare_only=True, sem=dma_sems[q])
                    nc.gpsimd.trigger_dma(count=None, queue_num=q)
                    qcnt[0] += 1
                    ioff += C * 8
                    toff = 0
                    for tt in range(GT):
                        J = int(jsg[tt, wdw])
                        if J == 0:
                            continue
                        src = Gt[:, toff:toff + J, :] \
                            .rearrange("p j f -> p f j")
                        nc.vector.tensor_reduce(
                            out=ssum[:, tt, sis[tt], :], in_=src, axis=X,
                            op=add)
                        nc.vector.tensor_reduce(
                            out=smax[:, tt, sis[tt], :], in_=src, axis=X,
                            op=mx)
                        sis[tt] += 1
                        toff += J

                for tt in range(GT):
                    si = sis[tt]
                    assert si > 0
                    tsum = red.tile([P, 64], dt.float32, tag="tsum")
                    tmax = red.tile([P, 64], dt.float32, tag="tmax")
                    nc.vector.tensor_reduce(
                        out=tsum[:], in_=ssum[:, tt, :si, :]
                            .rearrange("p s f -> p f s"), axis=X, op=add)
                    nc.vector.tensor_reduce(
                        out=tmax[:], in_=smax[:, tt, :si, :]
                            .rearrange("p s f -> p f s"), axis=X, op=mx)
                    # out = (sum * 1/K) - self ; out = max - self
                    nc.vector.scalar_tensor_tensor(
                        out=out_g[:, tt, 0:64], in0=tsum[:], scalar=1.0 / K,
                        in1=self_g[:, tt, :], op0=mult, op1=sub)
                    nc.vector.scalar_tensor_tensor(
                        out=out_g[:, tt, 64:128], in0=tmax[:], scalar=1.0,
                        in1=self_g[:, tt, :], op0=mult, op1=sub)

                nc.sync.dma_start(
                    out=out01[g * GT * P:(g + 1) * GT * P,
                              out_lo:out_lo + 128]
                        .rearrange("(m p) f -> p m f", p=P),
                    in_=out_g[:])

                if produce_f1:
                    f1g = red.tile([P, GT, 64], dt.float32, tag="f1g")
                    fsg = red.tile([P, GT, 64], dt.float32, tag="fsg")
                    for tt in range(GT):
                        tps = pst.tile([P, P], dt.float32, tag="tps")
                        nc.tensor.transpose(out=tps[:], in_=out_g[:, tt, :],
                                            identity=ident[:])
                        o0T = red.tile([P, P], dt.float32, tag="o0T")
                        nc.scalar.activation(out=o0T[:], in_=tps[:], func=Copy)
                        p1 = ps1.tile([P, 64], dt.float32, tag="p1")
                        nc.tensor.matmul(out=p1[:], lhsT=o0T[:], rhs=w1s[:],
                                         start=True, stop=True)
                        nc.scalar.activation(out=f1g[:, tt, :], in_=p1[:],
                                             func=Relu)
                        nc.scalar.activation(out=fsg[:, tt, :], in_=p1[:],
                                             func=Relu, scale=1.0 / W_EXP)
                    nc.sync.dma_start(
                        out=f1in[g * GT * P:(g + 1) * GT * P, :]
                            .rearrange("(m p) f -> p m f", p=P),
                        in_=f1g[:])
                    nc.sync.dma_start(
                        out=f1self[g * GT * P:(g + 1) * GT * P, :]
                            .rearrange("(m p) f -> p m f", p=P),
                        in_=fsg[:])

            tabs0 = [f0w[w][:, :] for w in range(NCORES)]
            for g in range(cfg.groups):
                gather_group(g, tabs0, f0self, 0, True)

            nc.gpsimd.collective_compute(
                "AllGather", mybir.AluOpType.bypass,
                replica_groups=[list(range(NCORES))],
                ins=[f1in[:].opt()], outs=[f1full[:].opt()])

            tabs1 = [f1full[w * SP:(w + 1) * SP, :] for w in range(NCORES)]
            for g in range(cfg.groups):
                gather_group(g, tabs1, f1self, 128, False)

    nc.finalize()
    return nc


def run(cfg: Cfg, inputs, trace=False):
    in_maps, js, goffs, perms = prep(cfg, **inputs)
    nc = build_nc(cfg, js, goffs)
    res = run_bass_kernel_spmd(nc, in_maps, core_ids=list(range(NCORES)),
                               trace=trace)
    x = np.asarray(inputs["x"], np.float32)
    S = cfg.shard
    out = np.empty((cfg.n, 320), np.float32)
    for c in range(NCORES):
        o = res.results[c]["out01"][:S]  # sorted order
        blk = out[c * S:(c + 1) * S]
        blk[perms[c], 0:128] = o[:, 0:128]
        blk[perms[c], 128:256] = o[:, 128:256]
    out[:, 256:320] = x
    return out, res


def kernel(x, neighbour_indices, W0, b0, W1, b1):
    cfg = Cfg(n=200000, k=32, f=64)
    out, _ = run(cfg, dict(x=x, neighbour_indices=neighbour_indices,
                           W0=W0, b0=b0, W1=W1, b1=b1))
    return out


# revision 11
# speedup vs baseline: 2.1245x; 2.1245x over previous
"""Trainium2 Bass kernel: 2-layer KNN message passing (AccumulateKnn MLP).

out = concat([L0, L1, x]); per layer: f = relu(feats@W+b) (b==0 required),
L = concat(w*mean_k f[idx], w*max_k f[idx]) - concat(f, f), w = exp(-1).

Design (8 cores, SPMD, full inputs fed per core):
- Vertices sharded (25000/core) and host-sorted per shard by window profile
  to reduce gather padding; outputs un-permuted on host.
- Feature tables laid out as 8 rank blocks ("windows") of SHARD_PAD rows.
  Layer-1 tables f0w[w] are computed redundantly on every core from a
  host-transposed, host-permuted x_T (per-window DRAM tiles so gathers can
  begin before the full dense phase finishes). Layer-2 table f1full is
  AllGathered from per-core shards.
- Gather calls are merged across GT=4 vertex tiles: one gpsimd.dma_gather
  per (group, window) rotating 4 SWDGE queues. This amortizes the ~1-2.4us
  fixed SWDGE descriptor-generation cost per call that dominated the
  per-(tile, window) version (GpSimd engine was 84% busy).
- Reduction per (tile, window): one strided DVE tensor_reduce (axis X over
  the J gathered columns) into dense per-tile slots, then one final reduce
  over slots, then scalar_tensor_tensor fuses the 1/K mean scale with the
  self-feature subtraction. PE does the layer-2 dense via transpose+matmul.
"""

import numpy as np

import concourse.bacc as bacc
import concourse.tile as tile
from concourse import mybir
from concourse.bass_utils import run_bass_kernel_spmd
from concourse.masks import make_identity

W_EXP = float(np.exp(-1.0))
NCORES = 8
P = 128
GA = 512   # vertices per dense-phase group
GT = 2     # vertex tiles (of 128) per merged gather call group
dt = mybir.dt


def _ceil_to(x, m):
    return (x + m - 1) // m * m


class Cfg:
    def __init__(self, n=200000, k=32, f=64):
        assert f == 64 and k == 32
        self.n, self.k, self.f = n, k, f
        assert n % NCORES == 0
        self.shard = n // NCORES
        # rank block / window size; +1 guarantees zero pad rows exist
        self.spad = _ceil_to(self.shard + 1, GA)
        assert self.spad < 32768
        assert self.spad % (P * GT) == 0
        self.tiles = self.spad // P
        self.groups = self.tiles // GT
        self.npad = self.spad * NCORES


def prep(cfg: Cfg, x, neighbour_indices, W0, b0, W1, b1):
    """Host prep: per-core sort, merged per-(group, window) index streams,
    inputs. Returns (in_maps, js, goffs, perms)."""
    assert np.allclose(b0, 0.0) and np.allclose(b1, 0.0), "nonzero bias unsupported"
    n, k, f, S, SP = cfg.n, cfg.k, cfg.f, cfg.shard, cfg.spad
    x = np.asarray(x, np.float32)
    idx = np.asarray(neighbour_indices, np.int64)
    w = np.float32(W_EXP)
    w0w = np.ascontiguousarray(np.asarray(W0, np.float32) * w)
    w1w = np.ascontiguousarray(np.asarray(W1, np.float32) * w)

    owner = (idx // S).astype(np.int32)  # window of each neighbour [n, k]
    local = (idx % S).astype(np.int32)

    # per-core sort of shard vertices by window profile (reduces padding):
    # (argmax window, its count, 2nd argmax window, its count)
    perms, invs = [], []
    for c in range(NCORES):
        m = np.zeros((S, NCORES), np.int32)
        ow = owner[c * S:(c + 1) * S]
        for wdw in range(NCORES):
            m[:, wdw] = (ow == wdw).sum(axis=1)
        a1 = m.argmax(1)
        mx1 = m.max(1)
        m2 = m.copy()
        m2[np.arange(S), a1] = -1
        a2 = m2.argmax(1)
        mx2 = m2.max(1)
        key = ((a1.astype(np.int64) * 64 + mx1) * 8 + a2) * 64 + mx2
        perm = np.argsort(key, kind="stable")  # sortedpos -> orig local
        inv = np.empty(S, np.int32)
        inv[perm] = np.arange(S, dtype=np.int32)
        perms.append(perm)
        invs.append(inv)

    # table-local position of each neighbour = inv-perm of its local id
    tloc = np.empty_like(local)
    for c in range(NCORES):
        sel = owner == c
        tloc[sel] = invs[c][local[sel]]

    # per-core sorted neighbour lists grouped by window (vectorized)
    PADIDX = SP - 1  # zero row
    sorted_t = np.zeros((NCORES, SP, k), np.int32)
    m_all = np.zeros((NCORES, SP, NCORES), np.int32)
    start_all = np.zeros((NCORES, SP, NCORES), np.int32)
    for c in range(NCORES):
        sl = slice(c * S, (c + 1) * S)
        ow_s = owner[sl][perms[c]]  # [S, k] sorted vertex order
        tl_s = tloc[sl][perms[c]]
        ord2 = np.argsort(ow_s, axis=1, kind="stable")
        sorted_t[c, :S] = np.take_along_axis(tl_s, ord2, axis=1)
        for wdw in range(NCORES):
            m_all[c, :S, wdw] = (ow_s == wdw).sum(axis=1)
        start_all[c] = np.concatenate(
            [np.zeros((SP, 1), np.int32), np.cumsum(m_all[c], axis=1)[:, :-1]],
            axis=1)

    # shared per-(tile, window) rectangle heights
    mt = m_all.reshape(NCORES, cfg.tiles, P, NCORES)
    js = mt.max(axis=2).max(axis=0)  # [tiles, NCORES]
    # per-call (group, window) column counts; carveout limit check
    jg = js.reshape(cfg.groups, GT, NCORES).sum(axis=1)  # [groups, NCORES]
    assert jg.max() * P <= 16000, f"gather call too large: {jg.max()*P} idxs"
    # int16-stream column offsets per group (all windows of a group adjacent)
    goffs = np.concatenate([[0], np.cumsum(jg.sum(axis=1) * 8)]).astype(np.int64)

    # build wrapped int16 streams [128, total_cols] per core, ordered
    # (group, window, tile-within-group)
    streams = []
    jmax = int(js.max())
    jr = np.arange(max(jmax, 1))
    for c in range(NCORES):
        blocks = []
        for g in range(cfg.groups):
            for wdw in range(NCORES):
                for tt in range(GT):
                    t = g * GT + tt
                    J = int(js[t, wdw])
                    if J == 0:
                        continue
                    r = slice(t * P, (t + 1) * P)
                    starts = start_all[c, r, wdw][:, None]  # [P, 1]
                    mm = m_all[c, r, wdw][:, None]
                    src = np.clip(starts + jr[None, :J], 0, k - 1)
                    vals = np.take_along_axis(sorted_t[c, r], src, axis=1)
                    vals = np.where(jr[None, :J] < mm, vals, PADIDX)  # [P, J]
                    flat = vals.T.reshape(J * P).astype(np.int16)
                    wr = flat.reshape(J * P // 16, 16).T  # [16, J*8]
                    blocks.append(np.tile(wr, (8, 1)))
        streams.append(np.ascontiguousarray(np.concatenate(blocks, axis=1)))
    assert streams[0].shape[1] == goffs[-1]

    # x_T in table order (all cores' sorted shards, zero-padded blocks)
    xt = np.zeros((f, cfg.npad), np.float32)
    for c in range(NCORES):
        xt[:, c * SP: c * SP + S] = x[c * S:(c + 1) * S][perms[c]].T

    in_maps = []
    for c in range(NCORES):
        xts = np.zeros((f, SP), np.float32)
        xts[:, :S] = x[c * S:(c + 1) * S][perms[c]].T
        in_maps.append({
            "xt": xt,
            "xts": np.ascontiguousarray(xts),
            "idxw": streams[c],
            "w0": w0w,
            "w1": w1w,
        })
    return in_maps, js, goffs, perms


def build_nc(cfg: Cfg, js, goffs):
    f, K, SP = cfg.f, cfg.k, cfg.spad
    total_cols = int(goffs[-1])
    jg = js.reshape(cfg.groups, GT, NCORES).sum(axis=1)
    nc = bacc.Bacc("TRN2", target_bir_lowering=False, debug=False,
                   enable_asserts=False, num_devices=NCORES,
                   num_swdge_queues=4)

    xt = nc.dram_tensor("xt", [f, cfg.npad], dt.float32, kind="ExternalInput")
    xts = nc.dram_tensor("xts", [f, SP], dt.float32, kind="ExternalInput")
    idxw = nc.dram_tensor("idxw", [P, total_cols], dt.int16, kind="ExternalInput")
    w0 = nc.dram_tensor("w0", [f, 64], dt.float32, kind="ExternalInput")
    w1 = nc.dram_tensor("w1", [128, 64], dt.float32, kind="ExternalInput")
    out01 = nc.dram_tensor("out01", [SP, 256], dt.float32, kind="ExternalOutput")

    qcnt = [0]
    add, mx, mult, sub = (mybir.AluOpType.add, mybir.AluOpType.max,
                          mybir.AluOpType.mult, mybir.AluOpType.subtract)
    X = mybir.AxisListType.X
    Relu = mybir.ActivationFunctionType.Relu
    Copy = mybir.ActivationFunctionType.Copy

    with tile.TileContext(nc) as tc:
        with (
            tc.tile_pool(name="dram", bufs=1, space="DRAM") as dram,
            tc.tile_pool(name="const", bufs=1) as const,
            tc.tile_pool(name="densea", bufs=3) as densea,
            tc.tile_pool(name="psa", bufs=2, space="PSUM") as psa,
            tc.tile_pool(name="gat", bufs=6) as gat,
            tc.tile_pool(name="idxp", bufs=3) as idxp,
            tc.tile_pool(name="red", bufs=3) as red,
            tc.tile_pool(name="pst", bufs=2, space="PSUM") as pst,
            tc.tile_pool(name="ps1", bufs=2, space="PSUM") as ps1,
        ):
            f0w = [dram.tile([SP, f], dt.float32, name=f"f0w{w}")
                   for w in range(NCORES)]
            f0self = dram.tile([SP, f], dt.float32, name="f0self")
            f1self = dram.tile([SP, f], dt.float32, name="f1self")
            f1in = dram.tile([SP, f], dt.float32, name="f1in")
            f1full = dram.tile([cfg.npad, f], dt.float32, name="f1full",
                               addr_space="Shared")

            w0s = const.tile([f, 64], dt.float32)
            nc.sync.dma_start(out=w0s[:], in_=w0[:, :])
            w1s = const.tile([128, 64], dt.float32)
            nc.sync.dma_start(out=w1s[:], in_=w1[:, :])
            ident = const.tile([P, P], dt.float32)
            make_identity(nc, ident[:])

            def dense_chunk(src, col0, n_cols, out_table, row0, scale):
                for g in range(n_cols // GA):
                    v0 = g * GA
                    xt_t = densea.tile([f, GA], dt.float32, tag="xa")
                    nc.sync.dma_start(out=xt_t[:],
                                      in_=src[:, col0 + v0: col0 + v0 + GA])
                    ps = psa.tile([P, 256], dt.float32, tag="psa")
                    for m in range(4):
                        nc.tensor.matmul(
                            out=ps[:, m * 64:(m + 1) * 64],
                            lhsT=xt_t[:, m * P:(m + 1) * P],
                            rhs=w0s[:],
                            start=True, stop=True)
                    fa = densea.tile([P, 4, 64], dt.float32, tag="fa")
                    nc.scalar.activation(
                        out=fa[:].rearrange("p a b -> p (a b)"), in_=ps[:],
                        func=Relu, scale=scale)
                    nc.sync.dma_start(
                        out=out_table[row0 + v0: row0 + v0 + GA, :]
                            .rearrange("(m p) f -> p m f", p=P),
                        in_=fa[:])

            # layer-1 dense: per-window tables so gathers can start early
            for wdw in range(NCORES):
                dense_chunk(xt, wdw * SP, SP, f0w[wdw], 0, 1.0)
            dense_chunk(xts, 0, SP, f0self, 0, 1.0 / W_EXP)

            def gather_group(g, tables, self_tab, out_lo, produce_f1):
                jsg = js[g * GT:(g + 1) * GT]  # [GT, NCORES]
                cg = int(jg[g].sum())          # total idx cols this group
                off0 = int(goffs[g])
                idx_t = idxp.tile([P, cg * 8], dt.int16, tag="idx")
                nc.sync.dma_start(out=idx_t[:], in_=idxw[:, off0:off0 + cg * 8])

                self_g = red.tile([P, GT, 64], dt.float32, tag="self")
                nc.sync.dma_start(
                    out=self_g[:],
                    in_=self_tab[g * GT * P:(g + 1) * GT * P, :]
                        .rearrange("(m p) f -> p m f", p=P))
                out_g = red.tile([P, GT, 128], dt.float32, tag="out")
                ssum = red.tile([P, GT, NCORES, 64], dt.float32, tag="ssum")
                smax = red.tile([P, GT, NCORES, 64], dt.float32, tag="smax")

                # one gather per window covering all GT tiles; its per-tile
                # reduces follow immediately so the G buffer recycles fast
                sis = [0] * GT
                ioff = 0
                for wdw in range(NCORES):
                    C = int(jg[g, wdw])
                    if C == 0:
                        continue
                    Gt = gat.tile([P, C, f], dt.float32, tag="G")
                    nidx = C * P
                    nc.gpsimd.dma_gather(
                        out_ap=Gt[:],
                        in_ap=tables[wdw],
                        idxs_ap=idx_t[:, ioff:ioff + C * 8],
                        num_idxs=nidx, num_idxs_reg=nidx, elem_size=f,
                        single_packet=False, queue_num=qcnt[0] % 4)
                    qcnt[0] += 1
                    ioff += C * 8
                    toff = 0
                    for tt in range(GT):
                        J = int(jsg[tt, wdw])
                        if J == 0:
                            continue
                        src = Gt[:, toff:toff + J, :] \
                            .rearrange("p j f -> p f j")
                        nc.vector.tensor_reduce(
                            out=ssum[:, tt, sis[tt], :], in_=src, axis=X,
                            op=add)
                        nc.vector.tensor_reduce(
                            out=smax[:, tt, sis[tt], :], in_=src, axis=X,
                            op=mx)
                        sis[tt] += 1
                        toff += J

                for tt in range(GT):
                    si = sis[tt]
                    assert si > 0
                    tsum = red.tile([P, 64], dt.float32, tag="tsum")
                    tmax = red.tile([P, 64], dt.float32, tag="tmax")
                    nc.vector.tensor_reduce(
                        out=tsum[:], in_=ssum[:, tt, :si, :]
                            .rearrange("p s f -> p f s"), axis=X, op=add)
                    nc.vector.tensor_reduce(
                        out=tmax[:], in_=smax[:, tt, :si, :]
                            .rearrange("p s f -> p f s"), axis=X, op=mx)
                    # out = (sum * 1/K) - self ; out = max - self
                    nc.vector.scalar_tensor_tensor(
                        out=out_g[:, tt, 0:64], in0=tsum[:], scalar=1.0 / K,
                        in1=self_g[:, tt, :], op0=mult, op1=sub)
                    nc.vector.scalar_tensor_tensor(
                        out=out_g[:, tt, 64:128], in0=tmax[:], scalar=1.0,
                        in1=self_g[:, tt, :], op0=mult, op1=sub)

                nc.sync.dma_start(
                    out=out01[g * GT * P:(g + 1) * GT * P,
                              out_lo:out_lo + 128]
                        .rearrange("(m p) f -> p m f", p=P),
                    in_=out_g[:])

                if produce_f1:
                    f1g = red.tile([P, GT, 64], dt.float32, tag="f1g")
                    fsg = red.tile([P, GT, 64], dt.float32, tag="fsg")
                    for tt in range(GT):
                        tps = pst.tile([P, P], dt.float32, tag="tps")
                        nc.tensor.transpose(out=tps[:], in_=out_g[:, tt, :],
                                            identity=ident[:])
                        o0T = red.tile([P, P], dt.float32, tag="o0T")
                        nc.scalar.activation(out=o0T[:], in_=tps[:], func=Copy)
                        p1 = ps1.tile([P, 64], dt.float32, tag="p1")
                        nc.tensor.matmul(out=p1[:], lhsT=o0T[:], rhs=w1s[:],
                                         start=True, stop=True)
                        nc.scalar.activation(out=f1g[:, tt, :], in_=p1[:],
                                             func=Relu)
                        nc.scalar.activation(out=fsg[:, tt, :], in_=p1[:],
                                             func=Relu, scale=1.0 / W_EXP)
                    nc.sync.dma_start(
                        out=f1in[g * GT * P:(g + 1) * GT * P, :]
                            .rearrange("(m p) f -> p m f", p=P),
                        in_=f1g[:])
                    nc.sync.dma_start(
                        out=f1self[g * GT * P:(g + 1) * GT * P, :]
                            .rearrange("(m p) f -> p m f", p=P),
                        in_=fsg[:])

            tabs0 = [f0w[w][:, :] for w in range(NCORES)]
            for g in range(cfg.groups):
                gather_group(g, tabs0, f0self, 0, True)

            nc.gpsimd.collective_compute(
                "AllGather", mybir.AluOpType.bypass,
                replica_groups=[list(range(NCORES))],
                ins=[f1in[:].opt()], outs=[f1full[:].opt()])

            tabs1 = [f1full[w * SP:(w + 1) * SP, :] for w in range(NCORES)]
            for g in range(cfg.groups):
                gather_group(g, tabs1, f1self, 128, False)

    nc.finalize()
    return nc


def run(cfg: Cfg, inputs, trace=False):
    in_maps, js, goffs, perms = prep(cfg, **inputs)
    nc = build_nc(cfg, js, goffs)
    res = run_bass_kernel_spmd(nc, in_maps, core_ids=list(range(NCORES)),
                               trace=trace)
    x = np.asarray(inputs["x"], np.float32)
    S = cfg.shard
    out = np.empty((cfg.n, 320), np.float32)
    for c in range(NCORES):
        o = res.results[c]["out01"][:S]  # sorted order
        blk = out[c * S:(c + 1) * S]
        blk[perms[c], 0:128] = o[:, 0:128]
        blk[perms[c], 128:256] = o[:, 128:256]
    out[:, 256:320] = x
    return out, res


def kernel(x, neighbour_indices, W0, b0, W1, b1):
    cfg = Cfg(n=200000, k=32, f=64)
    out, _ = run(cfg, dict(x=x, neighbour_indices=neighbour_indices,
                           W0=W0, b0=b0, W1=W1, b1=b1))
    return out


# revision 13
# speedup vs baseline: 2.5062x; 1.1796x over previous
"""Trainium2 Bass kernel: 2-layer KNN message passing (AccumulateKnn MLP).

out = concat([L0, L1, x]); per layer: f = relu(feats@W+b) (b==0 required),
L = concat(w*mean_k f[idx], w*max_k f[idx]) - concat(f, f), w = exp(-1).

Design (8 cores, SPMD, full inputs fed per core):
- Vertices sharded (25000/core) and host-sorted per shard to reduce gather
  padding; outputs un-permuted on host.
- Feature tables are laid out as 8 rank blocks of SHARD_PAD rows (one per
  core's sorted shard + zero pad rows). Layer-1 table f0t = w*relu(x@W0) is
  computed redundantly on every core from a host-transposed, host-permuted
  x_T. Layer-2 table f1full is AllGathered from per-core shards, which
  makes each rank block a gather "window" of SHARD_PAD (<32768) rows so the
  int16-indexed gpsimd dma_gather ucode can address it.
- Per 128-vertex tile: up to 8 dma_gather calls (one per window, rotating
  the 4 SWDGE queues for parallel descriptor generation), host-padded to a
  per-(tile,window) rectangle with pads pointing at zero rows (neutral for
  both sum and max of relu values). DVE pairwise trees reduce sum/max; ACT
  applies the 1/32 mean scale and psum evacuations; PE does the layer-2
  dense via transpose+matmul.
"""

import numpy as np

import concourse.bacc as bacc
import concourse.tile as tile
from concourse import mybir
from concourse.bass_utils import run_bass_kernel_spmd
from concourse.masks import make_identity

W_EXP = float(np.exp(-1.0))
NCORES = 8
P = 128
GA = 512  # vertices per dense-phase group
dt = mybir.dt
PHASES = "abgc"  # debug: subset of a(dense) b(layer1) g(allgather) c(layer2)


def _ceil_to(x, m):
    return (x + m - 1) // m * m


class Cfg:
    def __init__(self, n=200000, k=32, f=64):
        assert f == 64 and k == 32
        self.n, self.k, self.f = n, k, f
        assert n % NCORES == 0
        self.shard = n // NCORES
        # rank block / window size; +1 guarantees zero pad rows exist
        self.spad = _ceil_to(self.shard + 1, GA)
        assert self.spad < 32768
        assert self.spad % P == 0
        self.tiles = self.spad // P
        self.npad = self.spad * NCORES


def prep(cfg: Cfg, x, neighbour_indices, W0, b0, W1, b1):
    """Host prep: per-core sort, window index streams, inputs. Returns
    (in_maps, js, coffs, perms)."""
    assert np.allclose(b0, 0.0) and np.allclose(b1, 0.0), "nonzero bias unsupported"
    n, k, f, S, SP = cfg.n, cfg.k, cfg.f, cfg.shard, cfg.spad
    x = np.asarray(x, np.float32)
    idx = np.asarray(neighbour_indices, np.int64)
    w = np.float32(W_EXP)
    w0w = np.ascontiguousarray(np.asarray(W0, np.float32) * w)
    w1w = np.ascontiguousarray(np.asarray(W1, np.float32) * w)

    owner = (idx // S).astype(np.int32)  # window of each neighbour [n, k]
    local = (idx % S).astype(np.int32)

    # per-core sort of shard vertices by window profile (reduces padding):
    # key = (argmax window, its count, 2nd argmax window, its count)
    perms, invs = [], []
    for c in range(NCORES):
        m = np.zeros((S, NCORES), np.int32)
        ow = owner[c * S:(c + 1) * S]
        for wdw in range(NCORES):
            m[:, wdw] = (ow == wdw).sum(axis=1)
        a1 = m.argmax(1)
        mx1 = m.max(1)
        m2 = m.copy()
        m2[np.arange(S), a1] = -1
        a2 = m2.argmax(1)
        mx2 = m2.max(1)
        key = ((a1.astype(np.int64) * 64 + mx1) * 8 + a2) * 64 + mx2
        perm = np.argsort(key, kind="stable")  # sortedpos -> orig local
        inv = np.empty(S, np.int32)
        inv[perm] = np.arange(S, dtype=np.int32)
        perms.append(perm)
        invs.append(inv)

    # table-local position of each neighbour = inv-perm of its local id
    tloc = np.empty_like(local)
    for c in range(NCORES):
        sel = owner == c
        tloc[sel] = invs[c][local[sel]]

    # per-core sorted neighbour lists grouped by window (vectorized)
    PADIDX = SP - 1  # zero row
    sorted_t = np.zeros((NCORES, SP, k), np.int32)
    m_all = np.zeros((NCORES, SP, NCORES), np.int32)
    start_all = np.zeros((NCORES, SP, NCORES), np.int32)
    for c in range(NCORES):
        sl = slice(c * S, (c + 1) * S)
        ow_s = owner[sl][perms[c]]  # [S, k] sorted vertex order
        tl_s = tloc[sl][perms[c]]
        ord2 = np.argsort(ow_s, axis=1, kind="stable")
        sorted_t[c, :S] = np.take_along_axis(tl_s, ord2, axis=1)
        for wdw in range(NCORES):
            m_all[c, :S, wdw] = (ow_s == wdw).sum(axis=1)
        start_all[c] = np.concatenate(
            [np.zeros((SP, 1), np.int32), np.cumsum(m_all[c], axis=1)[:, :-1]],
            axis=1)

    # shared per-(tile, window) rectangle heights
    mt = m_all.reshape(NCORES, cfg.tiles, P, NCORES)
    js = mt.max(axis=2).max(axis=0)  # [tiles, NCORES]
    coffs = np.concatenate([[0], np.cumsum((js.sum(axis=1)) * 8)])  # int16 cols

    # build wrapped int16 streams [128, total_cols] per core
    streams = []
    jr = np.arange(int(js.max()) if js.max() > 0 else 1)
    for c in range(NCORES):
        blocks = []
        for t in range(cfg.tiles):
            r = slice(t * P, (t + 1) * P)
            for wdw in range(NCORES):
                J = int(js[t, wdw])
                if J == 0:
                    continue
                starts = start_all[c, r, wdw][:, None]  # [P, 1]
                mm = m_all[c, r, wdw][:, None]
                src = np.clip(starts + jr[None, :J], 0, k - 1)
                vals = np.take_along_axis(sorted_t[c, r], src, axis=1)
                vals = np.where(jr[None, :J] < mm, vals, PADIDX)  # [P, J]
                flat = vals.T.reshape(J * P).astype(np.int16)
                wr = flat.reshape(J * P // 16, 16).T  # [16, J*8]
                blocks.append(np.tile(wr, (8, 1)))
        streams.append(np.ascontiguousarray(np.concatenate(blocks, axis=1)))

    # x_T in table order (all cores' sorted shards, zero-padded blocks)
    xt = np.zeros((f, cfg.npad), np.float32)
    for c in range(NCORES):
        xt[:, c * SP: c * SP + S] = x[c * S:(c + 1) * S][perms[c]].T

    in_maps = []
    for c in range(NCORES):
        xts = np.zeros((f, SP), np.float32)
        xts[:, :S] = x[c * S:(c + 1) * S][perms[c]].T
        in_maps.append({
            "xt": xt,
            "xts": np.ascontiguousarray(xts),
            "idxw": streams[c],
            "w0": w0w,
            "w1": w1w,
        })
    return in_maps, js, coffs, perms


def build_nc(cfg: Cfg, js, coffs):
    f, K, SP = cfg.f, cfg.k, cfg.spad
    total_cols = int(coffs[-1])
    nc = bacc.Bacc("TRN2", target_bir_lowering=False, debug=False,
                   enable_asserts=False, num_devices=NCORES,
                   num_swdge_queues=4)

    xt = nc.dram_tensor("xt", [f, cfg.npad], dt.float32, kind="ExternalInput")
    xts = nc.dram_tensor("xts", [f, SP], dt.float32, kind="ExternalInput")
    idxw = nc.dram_tensor("idxw", [P, total_cols], dt.int16, kind="ExternalInput")
    w0 = nc.dram_tensor("w0", [f, 64], dt.float32, kind="ExternalInput")
    w1 = nc.dram_tensor("w1", [128, 64], dt.float32, kind="ExternalInput")
    out01 = nc.dram_tensor("out01", [SP, 256], dt.float32, kind="ExternalOutput")

    qcnt = [0]

    with tile.TileContext(nc) as tc:
        with (
            tc.tile_pool(name="dram", bufs=1, space="DRAM") as dram,
            tc.tile_pool(name="const", bufs=1) as const,
            tc.tile_pool(name="densea", bufs=3) as densea,
            tc.tile_pool(name="psa", bufs=2, space="PSUM") as psa,
            tc.tile_pool(name="gat", bufs=3) as gat,
            tc.tile_pool(name="red", bufs=3) as red,
            tc.tile_pool(name="pst", bufs=2, space="PSUM") as pst,
            tc.tile_pool(name="ps1", bufs=2, space="PSUM") as ps1,
        ):
            f0t = dram.tile([cfg.npad, f], dt.float32, name="f0t")
            f0self = dram.tile([SP, f], dt.float32, name="f0self")
            f1self = dram.tile([SP, f], dt.float32, name="f1self")
            f1in = dram.tile([SP, f], dt.float32, name="f1in")
            f1full = dram.tile([cfg.npad, f], dt.float32, name="f1full",
                               addr_space="Shared")

            w0s = const.tile([f, 64], dt.float32)
            nc.sync.dma_start(out=w0s[:], in_=w0[:, :])
            w1s = const.tile([128, 64], dt.float32)
            nc.sync.dma_start(out=w1s[:], in_=w1[:, :])
            ident = const.tile([P, P], dt.float32)
            make_identity(nc, ident[:])
            zero64 = const.tile([P, 64], dt.float32)
            nc.vector.memset(zero64[:], 0.0)

            def dense_phase(src, n_cols, out_table, scale, depth=4):
                for g in range(n_cols // GA):
                    v0 = g * GA
                    xt_t = densea.tile([f, GA], dt.float32, tag="xa")
                    nc.sync.dma_start(out=xt_t[:], in_=src[:, v0:v0 + GA])
                    if depth < 2:
                        continue
                    ps = psa.tile([P, 256], dt.float32, tag="psa")
                    for m in range(4):
                        nc.tensor.matmul(
                            out=ps[:, m * 64:(m + 1) * 64],
                            lhsT=xt_t[:, m * P:(m + 1) * P],
                            rhs=w0s[:],
                            start=True, stop=True)
                    if depth < 3:
                        continue
                    fa = densea.tile([P, 4, 64], dt.float32, tag="fa")
                    nc.scalar.activation(
                        out=fa[:].rearrange("p a b -> p (a b)"), in_=ps[:],
                        func=mybir.ActivationFunctionType.Relu, scale=scale)
                    if depth < 4:
                        continue
                    nc.sync.dma_start(
                        out=out_table[v0:v0 + GA, :].rearrange("(m p) f -> p m f", p=P),
                        in_=fa[:])

            if "a" in PHASES:
                depth = 4
                for tok in ("1", "2", "3"):
                    if tok in PHASES:
                        depth = int(tok)
                dense_phase(xt, cfg.npad, f0t, 1.0, depth)
                dense_phase(xts, SP, f0self, 1.0 / W_EXP, depth)

            def two_trees(G, T1, C):
                """Sum tree into T1, max tree in-place in G, over C col-groups
                of 64. Returns (S_ap, M_ap)."""
                add, mx = mybir.AluOpType.add, mybir.AluOpType.max
                tt = nc.vector.tensor_tensor
                if C == 1:
                    nc.vector.tensor_copy(out=T1[:, :64], in_=G[:, :64])
                    return T1[:, :64], G[:, :64]
                h, odd = C // 2, C % 2
                tt(out=T1[:, :h * 64], in0=G[:, :h * 64],
                   in1=G[:, h * 64:2 * h * 64], op=add)
                if odd:
                    tt(out=T1[:, :64], in0=T1[:, :64],
                       in1=G[:, 2 * h * 64:C * 64], op=add)
                tt(out=G[:, :h * 64], in0=G[:, :h * 64],
                   in1=G[:, h * 64:2 * h * 64], op=mx)
                if odd:
                    tt(out=G[:, :64], in0=G[:, :64],
                       in1=G[:, 2 * h * 64:C * 64], op=mx)
                C = h
                while C > 1:
                    if C % 2:
                        tt(out=T1[:, :64], in0=T1[:, :64],
                           in1=T1[:, (C - 1) * 64:C * 64], op=add)
                        tt(out=G[:, :64], in0=G[:, :64],
                           in1=G[:, (C - 1) * 64:C * 64], op=mx)
                        C -= 1
                    else:
                        h = C // 2
                        tt(out=T1[:, :h * 64], in0=T1[:, :h * 64],
                           in1=T1[:, h * 64:C * 64], op=add)
                        tt(out=G[:, :h * 64], in0=G[:, :h * 64],
                           in1=G[:, h * 64:C * 64], op=mx)
                        C = h
                return T1[:, :64], G[:, :64]

            cmax = int(js.sum(axis=1).max())

            def gather_reduce(t, table, self_tab, out_lo, produce_f1):
                C = int(js[t].sum())
                if C == 0:  # all-pad tile: only keep f1in zeroed for the AG
                    if produce_f1:
                        nc.gpsimd.dma_start(out=f1in[t * P:(t + 1) * P, :],
                                            in_=zero64[:])
                    return
                cols = C * 8
                off = int(coffs[t])
                idx_t = gat.tile([P, cols], dt.int16, tag="idx")
                nc.sync.dma_start(out=idx_t[:], in_=idxw[:, off:off + cols])
                G = gat.tile([P, C, f], dt.float32, tag="G")
                col = 0
                ioff = 0
                for wdw in range(NCORES):
                    J = int(js[t, wdw])
                    if J == 0:
                        continue
                    nidx = J * P
                    nc.gpsimd.dma_gather(
                        out_ap=G[:, col:col + J, :],
                        in_ap=table[wdw * SP:(wdw + 1) * SP, :],
                        idxs_ap=idx_t[:, ioff:ioff + J * 8],
                        num_idxs=nidx, num_idxs_reg=nidx, elem_size=f,
                        single_packet=False, queue_num=qcnt[0] % 4)
                    qcnt[0] += 1
                    col += J
                    ioff += J * 8
                Gv = G[:].rearrange("p c f -> p (c f)")
                T1 = red.tile([P, (cmax // 2 + 1) * 64], dt.float32, tag="T1")
                S_ap, M_ap = two_trees(Gv, T1[:], C)
                self_t = red.tile([P, 64], dt.float32, tag="self")
                nc.sync.dma_start(out=self_t[:], in_=self_tab[t * P:(t + 1) * P, :])
                meant = red.tile([P, 64], dt.float32, tag="meant")
                nc.scalar.activation(out=meant[:], in_=S_ap,
                                     func=mybir.ActivationFunctionType.Copy,
                                     scale=1.0 / K)
                out_t = red.tile([P, 128], dt.float32, tag="out")
                nc.vector.tensor_tensor(out=out_t[:, 0:64], in0=meant[:],
                                        in1=self_t[:], op=mybir.AluOpType.subtract)
                nc.vector.tensor_tensor(out=out_t[:, 64:128], in0=M_ap,
                                        in1=self_t[:], op=mybir.AluOpType.subtract)
                nc.sync.dma_start(
                    out=out01[t * P:(t + 1) * P, out_lo:out_lo + 128], in_=out_t[:])
                if produce_f1:
                    tps = pst.tile([P, P], dt.float32, tag="tps")
                    nc.tensor.transpose(out=tps[:], in_=out_t[:], identity=ident[:])
                    o0T = red.tile([P, P], dt.float32, tag="o0T")
                    nc.scalar.activation(out=o0T[:], in_=tps[:],
                                         func=mybir.ActivationFunctionType.Copy)
                    p1 = ps1.tile([P, 64], dt.float32, tag="p1")
                    nc.tensor.matmul(out=p1[:], lhsT=o0T[:], rhs=w1s[:],
                                     start=True, stop=True)
                    f1t = red.tile([P, 64], dt.float32, tag="f1t")
                    nc.scalar.activation(out=f1t[:], in_=p1[:],
                                         func=mybir.ActivationFunctionType.Relu)
                    fst = red.tile([P, 64], dt.float32, tag="fst")
                    nc.scalar.activation(out=fst[:], in_=p1[:],
                                         func=mybir.ActivationFunctionType.Relu,
                                         scale=1.0 / W_EXP)
                    nc.gpsimd.dma_start(out=f1in[t * P:(t + 1) * P, :], in_=f1t[:])
                    nc.sync.dma_start(out=f1self[t * P:(t + 1) * P, :], in_=fst[:])

            if "b" in PHASES:
                for t in range(cfg.tiles):
                    gather_reduce(t, f0t, f0self, 0, True)

            if "g" in PHASES:
                nc.gpsimd.collective_compute(
                    "AllGather", mybir.AluOpType.bypass,
                    replica_groups=[list(range(NCORES))],
                    ins=[f1in[:].opt()], outs=[f1full[:].opt()])

            if "c" in PHASES:
                for t in range(cfg.tiles):
                    gather_reduce(t, f1full, f1self, 128, False)

    nc.finalize()
    return nc


def run(cfg: Cfg, inputs, trace=False):
    in_maps, js, coffs, perms = prep(cfg, **inputs)
    nc = build_nc(cfg, js, coffs)
    res = run_bass_kernel_spmd(nc, in_maps, core_ids=list(range(NCORES)),
                               trace=trace)
    x = np.asarray(inputs["x"], np.float32)
    S = cfg.shard
    out = np.empty((cfg.n, 320), np.float32)
    for c in range(NCORES):
        o = res.results[c]["out01"][:S]  # sorted order
        blk = out[c * S:(c + 1) * S]
        blk[perms[c], 0:128] = o[:, 0:128]
        blk[perms[c], 128:256] = o[:, 128:256]
    out[:, 256:320] = x
    return out, res


def kernel(x, neighbour_indices, W0, b0, W1, b1):
    cfg = Cfg(n=200000, k=32, f=64)
    out, _ = run(cfg, dict(x=x, neighbour_indices=neighbour_indices,
                           W0=W0, b0=b0, W1=W1, b1=b1))
    return out
